# revision 42
# baseline (speedup 1.0000x reference)
"""Fp8 per-token/per-channel quantized linear for Trainium2, 8 NeuronCores.

Computation (matches the jax reference):
    amax[m]  = max_k |x[m, k]|                       (x is bf16)
    xs[m]    = max(amax, 1e-10) / 448
    x_q      = e4m3fn_round(x / xs)                  (values up to +-448)
    out      = bf16((x_q @ W^T) * xs * w_scales) + bf16(bias)

Mapping to TRN2 hardware:
  * TRN's fp8 E4M3 saturates at +-240 (256..448 are Inf/NaN), so we quantize
    at HALF scale: x_q' = e4m3_round(x * (224/amax)) == x_q / 2 exactly (the
    fp8 grid is self-similar under powers of two), and fold the factor 2 into
    the output scale: out = psum * (amax/224) * w_scales.  The reference
    weights are already exactly fp8-representable, so casting them is lossless.
  * Sharding: row-parallel over M (8 cores x 1024 rows).  Each core quantizes
    only its own rows, and streams the full weight, transposed on host to
    [K, N] tile layout and losslessly re-encoded to fp8.
  * x_q is transposed on-chip into [K, M] layout with PE transpose matmuls
    (contraction must sit on partitions for both matmul operands).
  * Main GEMM runs in fp8 with perf_mode=DoubleRow (k=256 per matmul).

Schedule: the kernel is PE-bound (DoubleRow GEMM ~221us + PE transposes
~19us; measured ~270us vs the 300us baseline).  Phase 1 interleaves, per
128-row tile: DVE amax (f32 reduce halves; no fast DVE mode exists for
reduce), a tiny DVE scale chain, ACT quant copy (halves), 32 PE transpose
matmuls (is_transpose mode; fp8 PSUM with 2-byte element step), 4 ACT
psum evicts, and two N-blocks of GEMM (the second deferred by one tile),
which keeps the PE saturated (~9.3us/tile) above the producer rate
(ACT ~8.2us, DVE ~5.8us); phase 2 is pure gapless GEMM.  The epilogue is
one fused DVE scalar_tensor_tensor (psum*xs[m]*ws[n] -> bf16) + bias add
(gpsimd in phase 1, DVE in phase 2).

DMA lessons baked in: DGE rings retire in order and doorbells block the
issuing queue when the ring fills, so the ACT queue must stay nearly
DMA-free (a blocked doorbell stalls the quant copies behind it); DMA
engines round-robin between queues per ~4-16KB descriptor, so weight
slabs load as 4KB-run quarters to not starve the 8KB-run x rows; ws/bias
load as 16KB rows and partition-broadcast on the idle gpsimd engine.
Soft scheduler deps (sync=False) are best-effort; hard deps can bake a
same-queue deadlock, so cross-tile ordering relies on issue order plus
soft edges only.
"""

import os
import numpy as np
import ml_dtypes
from contextlib import ExitStack

import concourse.bass as bass
import concourse.bacc as bacc
import concourse.tile as tile
from concourse import mybir
from concourse.bass_utils import run_bass_kernel_spmd
from concourse.masks import make_identity

P = 128
M, K, N = 8192, 4096, 4096
NCORES = 8
M_SHARD = M // NCORES          # 1024 rows of x per core
M_TILES = M_SHARD // P         # 8
K_SUBS = K // P                # 32
K_SUPERS = K // (2 * P)        # 16 (DoubleRow consumes 256 rows of K)
KH = K // 2                    # 2048, half-tile for split reduces
N_BLK = 512
N_BLKS = N // N_BLK            # 8
NB_PHASE1 = 2                  # GEMM N-blocks interleaved into the quant loop

FP8 = mybir.dt.float8e4
F32 = mybir.dt.float32
BF16 = mybir.dt.bfloat16

USE_IS_TRANSPOSE = True

_PROGRAM_CACHE = {}


def _build_program():
    nc = bacc.Bacc(None, target_bir_lowering=False)

    x_d = nc.declare_dram_parameter("x", [M_SHARD, K], BF16, isOutput=False)
    # host layout: wt[nb, p, ksub, n] = weight[nb*512 + n, ksub*128 + p],
    # losslessly re-encoded to fp8 (reference weights are fp8-round-tripped)
    wt_d = nc.declare_dram_parameter("wt", [N_BLKS, P, K_SUPERS, 2, N_BLK], FP8, isOutput=False)
    ws_d = nc.declare_dram_parameter("ws", [N], F32, isOutput=False)
    bias_d = nc.declare_dram_parameter("bias", [N], F32, isOutput=False)
    out_d = nc.declare_dram_parameter("out", [M_SHARD, N], BF16, isOutput=True)

    x_ap = x_d[:]
    wt_ap = wt_d[:]
    out_ap = out_d[:]

    with tile.TileContext(nc) as tc, ExitStack() as ctx:
        singles = ctx.enter_context(tc.tile_pool(name="singles", bufs=1))
        xpool = ctx.enter_context(tc.tile_pool(name="xpool", bufs=3))
        xqpool = ctx.enter_context(tc.tile_pool(name="xqpool", bufs=2))
        stats = ctx.enter_context(tc.tile_pool(name="stats", bufs=4))
        xspool = ctx.enter_context(tc.tile_pool(name="xspool", bufs=M_TILES))
        xqtpool = ctx.enter_context(tc.tile_pool(name="xqtpool", bufs=M_TILES))
        wqpool = ctx.enter_context(tc.tile_pool(name="wqpool", bufs=12))
        opool = ctx.enter_context(tc.tile_pool(name="opool", bufs=4))
        psum_mm = ctx.enter_context(tc.tile_pool(name="psum_mm", bufs=6, space="PSUM"))
        psum_xs = ctx.enter_context(tc.tile_pool(name="psum_xs", bufs=2, space="PSUM"))

        # ---- upfront DMA issue: x tiles 0-1 on the sync ring; weight slabs
        # on the scalar ring (first two quartered); ws/bias broadcasts are
        # HBM-read-light and use the scalar ring's broadcast path.
        x_tiles = [None] * M_TILES

        def issue_x(mt, split=False):
            t = xpool.tile([P, K], BF16, tag="xt")
            if split:
                # first tile rides both DGE rings so the halves land in
                # parallel right after ring warmup
                nc.sync.dma_start(out=t[:, 0:KH], in_=x_ap[mt * P:(mt + 1) * P, 0:KH])
                nc.scalar.dma_start(out=t[:, KH:K], in_=x_ap[mt * P:(mt + 1) * P, KH:K])
            else:
                nc.sync.dma_start(out=t[:], in_=x_ap[mt * P:(mt + 1) * P, :])
            x_tiles[mt] = t

        wslab_tiles = [None] * N_BLKS

        def issue_wslab(nb, engine):
            # every slab loads as 4 quarter-DMAs: 4KB-per-partition runs keep
            # the round-robin DMA queues fair vs the 8KB x rows, and give the
            # GEMM quarter-granular deps on the arriving weights
            quarters = []
            for q in range(4):
                t = wqpool.tile([P, 4, 2, N_BLK], FP8, tag="wq")
                engine.dma_start(out=t[:], in_=wt_ap[nb][:, 4 * q:4 * q + 4, :, :])
                quarters.append(t)
            wslab_tiles[nb] = quarters

        def slab_rhs(nb, j):
            return wslab_tiles[nb][j // 4][:, j % 4, :, :]

        # DGE rings retire in order and doorbells block while the ring is
        # full, so small row-loads go first on the scalar ring; bulk loads
        # are ordered by deadline.  ws/bias broadcast on-chip via the idle
        # gpsimd engine instead of a slow 2MB DMA row-broadcast.
        # x0 loads as four quarter-DMAs alternating rings so its amax can
        # reduce quarter-by-quarter while the rest arrives
        KQ = K // 4
        xt0 = xpool.tile([P, K], BF16, tag="xt")
        for q, eng in enumerate((nc.sync, nc.scalar, nc.sync, nc.scalar)):
            eng.dma_start(out=xt0[:, q * KQ:(q + 1) * KQ],
                          in_=x_ap[0:P, q * KQ:(q + 1) * KQ])
        x_tiles[0] = xt0
        ws_row = singles.tile([1, N], F32)
        nc.scalar.dma_start(out=ws_row[:], in_=bass.AP(tensor=ws_d[:].tensor, offset=0, ap=[[0, 1], [1, N]]))
        bias_row = singles.tile([1, N], F32)
        nc.scalar.dma_start(out=bias_row[:], in_=bass.AP(tensor=bias_d[:].tensor, offset=0, ap=[[0, 1], [1, N]]))
        issue_wslab(1, nc.scalar)

        sq0 = [wqpool.tile([P, 4, 2, N_BLK], FP8, tag="wq", name=f"s0q{q}") for q in range(4)]
        for q in (0, 1):
            nc.sync.dma_start(out=sq0[q][:], in_=wt_ap[0][:, 4 * q:4 * q + 4, :, :])
        issue_x(1)
        for q in (2, 3):
            nc.sync.dma_start(out=sq0[q][:], in_=wt_ap[0][:, 4 * q:4 * q + 4, :, :])
        wslab_tiles[0] = sq0

        antident = singles.tile([P, P], F32)
        nc.gpsimd.memset(antident[:], 0.0)
        nc.gpsimd.affine_select(
            out=antident[:], in_=antident[:],
            compare_op=mybir.AluOpType.not_equal, fill=1.0,
            base=-(P - 1), pattern=[[1, P]], channel_multiplier=1,
        )
        ws_b = singles.tile([P, N], F32)
        nc.gpsimd.partition_broadcast(ws_b[:], ws_row[:], channels=P)
        bias_b = singles.tile([P, N], F32)
        nc.gpsimd.partition_broadcast(bias_b[:], bias_row[:], channels=P)

        xs_tiles = []
        xqt_tiles = []
        prev_inv_inst = None

        def epilogue(mt, nb, pm, phase1):
            # out = bf16(psum * xs[m] * ws[n]) + bias[n]; the fused
            # scalar_tensor_tensor keeps a single rounding to bf16.
            sb1 = opool.tile([P, N_BLK], BF16, tag="sb1")
            nc.vector.scalar_tensor_tensor(
                out=sb1[:], in0=pm[:], scalar=xs_tiles[mt][:],
                in1=ws_b[:, nb * N_BLK:(nb + 1) * N_BLK],
                op0=mybir.AluOpType.mult, op1=mybir.AluOpType.mult,
            )
            sb2 = opool.tile([P, N_BLK], BF16, tag="sb2")
            eng = nc.gpsimd if phase1 else nc.vector
            eng.tensor_add(sb2[:], sb1[:], bias_b[:, nb * N_BLK:(nb + 1) * N_BLK])
            nc.sync.dma_start(
                out=out_ap[mt * P:(mt + 1) * P, nb * N_BLK:(nb + 1) * N_BLK],
                in_=sb2[:],
            )

        def gemm_block(mt, nb, phase1=False, fix_xs=False):
            pm = psum_mm.tile([P, N_BLK], F32, tag="pm")
            xqt_f8 = xqt_tiles[mt].bitcast(FP8)
            for j in range(K_SUPERS):
                nc.tensor.matmul(
                    out=pm[:],
                    lhsT=xqt_f8[:, j, :],
                    rhs=slab_rhs(nb, j),
                    start=(j == 0), stop=(j == K_SUPERS - 1),
                    perf_mode=mybir.MatmulPerfMode.DoubleRowSwInterleave,
                )
                if j == 0 and fix_xs:
                    # SwInterleave reads stationary columns reversed, so psum
                    # partition m holds token 127-m; reverse xs to match via
                    # an anti-identity matmul.  Issued here because matmul
                    # j=0 already waited transitively on this tile's scale
                    # chain, so the tiny matmul can never head-of-line-block
                    # the PE (it cost 5-9us per tile in the producer block).
                    pxs = psum_xs.tile([P, 1], F32, tag="pxs")
                    nc.tensor.matmul(out=pxs[:], lhsT=antident[:],
                                     rhs=xs_tiles[mt][:], start=True, stop=True)
                    xs_rev = xspool.tile([P, 1], F32, tag="xsr")
                    nc.vector.tensor_copy(out=xs_rev[:], in_=pxs[:])
                    xs_tiles[mt] = xs_rev
            epilogue(mt, nb, pm, phase1)

        # ---- phase 1: per 128-row tile: quantize, transpose, and two
        # N-blocks of GEMM (keeps the PE saturated while later tiles
        # quantize).
        for mt in range(M_TILES):
            if mt + 2 < M_TILES:
                issue_x(mt + 2)
            if mt == 5:
                issue_wslab(2, nc.sync)

            xt = x_tiles[mt]
            nparts = 4 if mt == 0 else 2
            part_maxes = []
            reduces = []
            KP_ = K // nparts
            for h in range(nparts):
                pmx = stats.tile([P, 1], F32, tag=f"amax_{h}", bufs=2)
                r = nc.vector.tensor_reduce(
                    out=pmx[:], in_=xt[:, h * KP_:(h + 1) * KP_],
                    axis=mybir.AxisListType.X, op=mybir.AluOpType.max,
                    apply_absolute_value=True,
                )
                part_maxes.append(pmx)
                reduces.append(r)
            # order this tile's reduces after the previous tile's scale
            # chain so the 2.3us reduces don't delay the chain that gates
            # ACT quant
            if prev_inv_inst is not None:
                for r in reduces:
                    tile.add_dep_helper(r.ins, prev_inv_inst.ins, sync=False,
                                        reason="stats chain before next reduce")
            with tc.high_priority():
                while len(part_maxes) > 1:
                    nxt = []
                    for i in range(0, len(part_maxes), 2):
                        cm = stats.tile([P, 1], F32, tag=f"cmb{len(part_maxes)}_{i}",
                                        bufs=2)
                        nc.vector.tensor_max(cm[:], part_maxes[i][:],
                                             part_maxes[i + 1][:])
                        nxt.append(cm)
                    part_maxes = nxt
                amax = part_maxes[0]
                # xs = max(amax, eps) * (1/224); quant scale is exactly 1/xs
                xs = xspool.tile([P, 1], F32, tag="xs")
                nc.vector.tensor_scalar(
                    out=xs[:], in0=amax[:],
                    scalar1=1e-10, scalar2=1.0 / 224.0,
                    op0=mybir.AluOpType.max, op1=mybir.AluOpType.mult,
                )
                xs_tiles.append(xs)
                inv = stats.tile([P, 1], F32, tag="inv")
                prev_inv_inst = nc.vector.reciprocal(out=inv[:], in_=xs[:])

            # quantize in halves and XBAR-transpose each half as uint16
            # pairs on its own DGE ring (the XBAR is descriptor-bound:
            # ~1024 descriptors per half).  fp8 byte [p, t, 2m+e] =
            # xq[m, 256t+2p+e], consumed directly by DoubleRowSwInterleave
            # (weights k-order pre-matched on host) -- no PE transposes,
            # no psum evicts.
            xqt = xqtpool.tile([P, K_SUPERS, P], mybir.dt.uint16, tag="xqt")
            for h, eng in enumerate((nc.scalar, nc.sync)):
                xq_h = xqpool.tile([P, KH], FP8, tag=f"xqh{h}", bufs=2)
                nc.scalar.activation(
                    out=xq_h[:], in_=xt[:, h * KH:(h + 1) * KH],
                    func=mybir.ActivationFunctionType.Copy, scale=inv[:],
                )
                eng.dma_start(
                    out=xqt[:, 8 * h:8 * h + 8, :],
                    in_=xq_h.bitcast(mybir.dt.uint16)[:],
                    transpose=True,
                )
            xqt_tiles.append(xqt)

            if mt >= 1:
                gemm_block(mt - 1, 1, phase1=True)
            gemm_block(mt, 0, phase1=True, fix_xs=True)

        gemm_block(M_TILES - 1, 1, phase1=True)

        # ---- phase 2: pure fp8 DoubleRow GEMM over the remaining N-blocks
        for nb in range(NB_PHASE1, N_BLKS):
            if nb + 1 < N_BLKS:
                issue_wslab(nb + 1, nc.sync)
            for mt in range(M_TILES):
                gemm_block(mt, nb)

    nc.compile()
    return nc


def _get_program():
    if "nc" not in _PROGRAM_CACHE:
        _PROGRAM_CACHE["nc"] = _build_program()
    return _PROGRAM_CACHE["nc"]


def _run_sharded(x, weight, weight_scales, bias, trace=False):
    x = np.asarray(x).astype(ml_dtypes.bfloat16, copy=False)
    weight = np.asarray(weight, dtype=np.float32)
    weight_scales = np.asarray(weight_scales, dtype=np.float32)
    bias = np.asarray(bias, dtype=np.float32)

    # host-side sharding / layout only:
    # wt[nb, p, ksub, n] = weight[nb*512 + n, ksub*128 + p], re-encoded to
    # fp8 e4m3 (lossless: the reference weights are fp8-round-tripped values)
    wt = np.ascontiguousarray(
        weight.T.reshape(K_SUPERS, P, 2, N_BLKS, N_BLK).transpose(3, 1, 0, 2, 4)
    ).astype(ml_dtypes.float8_e4m3)
    in_maps = []
    for c in range(NCORES):
        in_maps.append({
            "x": np.ascontiguousarray(x[c * M_SHARD:(c + 1) * M_SHARD]),
            "wt": wt,
            "ws": weight_scales,
            "bias": bias,
        })

    nc = _get_program()
    res = run_bass_kernel_spmd(nc, in_maps, core_ids=list(range(NCORES)), trace=trace)
    out = np.concatenate([res.results[c]["out"] for c in range(NCORES)], axis=0)
    # psum rows are token-reversed per 128-row tile (SwInterleave
    # reversed-column convention): un-reverse on the host
    out = np.ascontiguousarray(out.reshape(-1, P, N)[:, ::-1, :].reshape(M, N))
    return out, res.exec_time_ns


def kernel(x, weight, weight_scales, bias):
    out, _ = _run_sharded(x, weight, weight_scales, bias,
                          trace=bool(os.environ.get("KERNEL_TRACE")))
    return out


# revision 43
# speedup vs baseline: 1.1021x; 1.1021x over previous
"""Fp8 per-token/per-channel quantized linear for Trainium2, 8 NeuronCores.

Computation (matches the jax reference):
    amax[m]  = max_k |x[m, k]|                       (x is bf16)
    xs[m]    = max(amax, 1e-10) / 448
    x_q      = e4m3fn_round(x / xs)                  (values up to +-448)
    out      = bf16((x_q @ W^T) * xs * w_scales) + bf16(bias)

Mapping to TRN2 hardware:
  * TRN's fp8 E4M3 saturates at +-240 (256..448 are Inf/NaN), so we quantize
    at HALF scale: x_q' = e4m3_round(x * (224/amax)) == x_q / 2 exactly (the
    fp8 grid is self-similar under powers of two), and fold the factor 2 into
    the output scale: out = psum * (amax/224) * w_scales.  The reference
    weights are already exactly fp8-representable, so casting them is lossless.
  * Sharding: row-parallel over M (8 cores x 1024 rows).  Each core quantizes
    only its own rows, and streams the full weight, transposed on host to
    [K, N] tile layout and losslessly re-encoded to fp8.
  * x_q is transposed on-chip into [K, M] layout with PE transpose matmuls
    (contraction must sit on partitions for both matmul operands).
  * Main GEMM runs in fp8 with perf_mode=DoubleRow (k=256 per matmul).

Schedule: the kernel is PE-bound (DoubleRow GEMM ~221us + PE transposes
~19us; measured ~270us vs the 300us baseline).  Phase 1 interleaves, per
128-row tile: DVE amax (f32 reduce halves; no fast DVE mode exists for
reduce), a tiny DVE scale chain, ACT quant copy (halves), 32 PE transpose
matmuls (is_transpose mode; fp8 PSUM with 2-byte element step), 4 ACT
psum evicts, and two N-blocks of GEMM (the second deferred by one tile),
which keeps the PE saturated (~9.3us/tile) above the producer rate
(ACT ~8.2us, DVE ~5.8us); phase 2 is pure gapless GEMM.  The epilogue is
one fused DVE scalar_tensor_tensor (psum*xs[m]*ws[n] -> bf16) + bias add
(gpsimd in phase 1, DVE in phase 2).

DMA lessons baked in: DGE rings retire in order and doorbells block the
issuing queue when the ring fills, so the ACT queue must stay nearly
DMA-free (a blocked doorbell stalls the quant copies behind it); DMA
engines round-robin between queues per ~4-16KB descriptor, so weight
slabs load as 4KB-run quarters to not starve the 8KB-run x rows; ws/bias
load as 16KB rows and partition-broadcast on the idle gpsimd engine.
Soft scheduler deps (sync=False) are best-effort; hard deps can bake a
same-queue deadlock, so cross-tile ordering relies on issue order plus
soft edges only.
"""

import os
import numpy as np
import ml_dtypes
from contextlib import ExitStack

import concourse.bass as bass
import concourse.bacc as bacc
import concourse.tile as tile
from concourse import mybir
from concourse.bass_utils import run_bass_kernel_spmd
from concourse.masks import make_identity

P = 128
M, K, N = 8192, 4096, 4096
NCORES = 8
M_SHARD = M // NCORES          # 1024 rows of x per core
M_TILES = M_SHARD // P         # 8
K_SUBS = K // P                # 32
K_SUPERS = K // (2 * P)        # 16 (DoubleRow consumes 256 rows of K)
KH = K // 2                    # 2048, half-tile for split reduces
N_BLK = 512
N_BLKS = N // N_BLK            # 8
NB_PHASE1 = 2                  # GEMM N-blocks interleaved into the quant loop

FP8 = mybir.dt.float8e4
F32 = mybir.dt.float32
BF16 = mybir.dt.bfloat16

USE_IS_TRANSPOSE = True

_PROGRAM_CACHE = {}


def _build_program():
    nc = bacc.Bacc(None, target_bir_lowering=False)

    x_d = nc.declare_dram_parameter("x", [M_SHARD, K], BF16, isOutput=False)
    # host layout: wt[nb, p, ksub, n] = weight[nb*512 + n, ksub*128 + p],
    # losslessly re-encoded to fp8 (reference weights are fp8-round-tripped)
    wt_d = nc.declare_dram_parameter("wt", [N_BLKS, P, K_SUBS, N_BLK], FP8, isOutput=False)
    ws_d = nc.declare_dram_parameter("ws", [N], F32, isOutput=False)
    bias_d = nc.declare_dram_parameter("bias", [N], F32, isOutput=False)
    out_d = nc.declare_dram_parameter("out", [M_SHARD, N], BF16, isOutput=True)

    x_ap = x_d[:]
    wt_ap = wt_d[:]
    out_ap = out_d[:]

    with tile.TileContext(nc) as tc, ExitStack() as ctx:
        singles = ctx.enter_context(tc.tile_pool(name="singles", bufs=1))
        xpool = ctx.enter_context(tc.tile_pool(name="xpool", bufs=3))
        xqpool = ctx.enter_context(tc.tile_pool(name="xqpool", bufs=2))
        stats = ctx.enter_context(tc.tile_pool(name="stats", bufs=4))
        xspool = ctx.enter_context(tc.tile_pool(name="xspool", bufs=M_TILES))
        xqtpool = ctx.enter_context(tc.tile_pool(name="xqtpool", bufs=M_TILES))
        wqpool = ctx.enter_context(tc.tile_pool(name="wqpool", bufs=12))
        opool = ctx.enter_context(tc.tile_pool(name="opool", bufs=4))
        psum_tr = ctx.enter_context(tc.tile_pool(name="psum_tr", bufs=2, space="PSUM"))
        psum_mm = ctx.enter_context(tc.tile_pool(name="psum_mm", bufs=4, space="PSUM"))

        # ---- upfront DMA issue: x tiles 0-1 on the sync ring; weight slabs
        # on the scalar ring (first two quartered); ws/bias broadcasts are
        # HBM-read-light and use the scalar ring's broadcast path.
        x_tiles = [None] * M_TILES

        def issue_x(mt, split=False):
            t = xpool.tile([P, K], BF16, tag="xt")
            if split:
                # first tile rides both DGE rings so the halves land in
                # parallel right after ring warmup
                nc.sync.dma_start(out=t[:, 0:KH], in_=x_ap[mt * P:(mt + 1) * P, 0:KH])
                nc.scalar.dma_start(out=t[:, KH:K], in_=x_ap[mt * P:(mt + 1) * P, KH:K])
            else:
                nc.sync.dma_start(out=t[:], in_=x_ap[mt * P:(mt + 1) * P, :])
            x_tiles[mt] = t

        wslab_tiles = [None] * N_BLKS

        def issue_wslab(nb, engine):
            # every slab loads as 4 quarter-DMAs: 4KB-per-partition runs keep
            # the round-robin DMA queues fair vs the 8KB x rows, and give the
            # GEMM quarter-granular deps on the arriving weights
            quarters = []
            for q in range(4):
                t = wqpool.tile([P, 8, N_BLK], FP8, tag="wq")
                engine.dma_start(out=t[:], in_=wt_ap[nb][:, 8 * q:8 * q + 8, :])
                quarters.append(t)
            wslab_tiles[nb] = quarters

        def slab_rhs(nb, j):
            jj = j % 4
            return wslab_tiles[nb][j // 4][:, 2 * jj:2 * jj + 2, :]

        # DGE rings retire in order and doorbells block while the ring is
        # full, so small row-loads go first on the scalar ring; bulk loads
        # are ordered by deadline.  ws/bias broadcast on-chip via the idle
        # gpsimd engine instead of a slow 2MB DMA row-broadcast.
        # x0 loads as four quarter-DMAs alternating rings so its amax can
        # reduce quarter-by-quarter while the rest arrives
        KQ = K // 4
        xt0 = xpool.tile([P, K], BF16, tag="xt")
        for q, eng in enumerate((nc.sync, nc.scalar, nc.sync, nc.scalar)):
            eng.dma_start(out=xt0[:, q * KQ:(q + 1) * KQ],
                          in_=x_ap[0:P, q * KQ:(q + 1) * KQ])
        x_tiles[0] = xt0
        ws_row = singles.tile([1, N], F32)
        nc.scalar.dma_start(out=ws_row[:], in_=bass.AP(tensor=ws_d[:].tensor, offset=0, ap=[[0, 1], [1, N]]))
        bias_row = singles.tile([1, N], F32)
        nc.scalar.dma_start(out=bias_row[:], in_=bass.AP(tensor=bias_d[:].tensor, offset=0, ap=[[0, 1], [1, N]]))
        issue_wslab(1, nc.scalar)

        sq0 = [wqpool.tile([P, 8, N_BLK], FP8, tag="wq", name=f"s0q{q}") for q in range(4)]
        for q in (0, 1):
            nc.sync.dma_start(out=sq0[q][:], in_=wt_ap[0][:, 8 * q:8 * q + 8, :])
        issue_x(1)
        for q in (2, 3):
            nc.sync.dma_start(out=sq0[q][:], in_=wt_ap[0][:, 8 * q:8 * q + 8, :])
        wslab_tiles[0] = sq0

        ident = singles.tile([P, P], FP8)
        make_identity(nc, ident)
        ws_b = singles.tile([P, N], F32)
        nc.gpsimd.partition_broadcast(ws_b[:], ws_row[:], channels=P)
        bias_b = singles.tile([P, N], F32)
        nc.gpsimd.partition_broadcast(bias_b[:], bias_row[:], channels=P)

        xs_tiles = []
        xqt_tiles = []
        prev_inv_inst = None

        def epilogue(mt, nb, pm, phase1):
            # out = bf16(psum * xs[m] * ws[n]) + bias[n]; the fused
            # scalar_tensor_tensor keeps a single rounding to bf16.
            sb1 = opool.tile([P, N_BLK], BF16, tag="sb1")
            nc.vector.scalar_tensor_tensor(
                out=sb1[:], in0=pm[:], scalar=xs_tiles[mt][:],
                in1=ws_b[:, nb * N_BLK:(nb + 1) * N_BLK],
                op0=mybir.AluOpType.mult, op1=mybir.AluOpType.mult,
            )
            sb2 = opool.tile([P, N_BLK], BF16, tag="sb2")
            eng = nc.gpsimd if phase1 else nc.vector
            eng.tensor_add(sb2[:], sb1[:], bias_b[:, nb * N_BLK:(nb + 1) * N_BLK])
            nc.sync.dma_start(
                out=out_ap[mt * P:(mt + 1) * P, nb * N_BLK:(nb + 1) * N_BLK],
                in_=sb2[:],
            )

        def gemm_block(mt, nb, phase1=False):
            pm = psum_mm.tile([P, N_BLK], F32, tag="pm")
            for j in range(K_SUPERS):
                g, jj = divmod(j, 4)
                nc.tensor.matmul(
                    out=pm[:],
                    lhsT=xqt_tiles[mt][g][:, 2 * jj:2 * jj + 2, :],
                    rhs=slab_rhs(nb, j),
                    start=(j == 0), stop=(j == K_SUPERS - 1),
                    perf_mode=mybir.MatmulPerfMode.DoubleRow,
                )
            epilogue(mt, nb, pm, phase1)

        # ---- phase 1: per 128-row tile: quantize, transpose, and two
        # N-blocks of GEMM (keeps the PE saturated while later tiles
        # quantize).
        for mt in range(M_TILES):
            if mt + 2 < M_TILES:
                issue_x(mt + 2)
            if mt == 5:
                issue_wslab(2, nc.sync)

            xt = x_tiles[mt]
            nparts = 4 if mt == 0 else 2
            part_maxes = []
            reduces = []
            KP_ = K // nparts
            for h in range(nparts):
                pmx = stats.tile([P, 1], F32, tag=f"amax_{h}", bufs=2)
                r = nc.vector.tensor_reduce(
                    out=pmx[:], in_=xt[:, h * KP_:(h + 1) * KP_],
                    axis=mybir.AxisListType.X, op=mybir.AluOpType.max,
                    apply_absolute_value=True,
                )
                part_maxes.append(pmx)
                reduces.append(r)
            # order this tile's reduces after the previous tile's scale
            # chain so the 2.3us reduces don't delay the chain that gates
            # ACT quant
            if prev_inv_inst is not None:
                for r in reduces:
                    tile.add_dep_helper(r.ins, prev_inv_inst.ins, sync=False,
                                        reason="stats chain before next reduce")
            with tc.high_priority():
                while len(part_maxes) > 1:
                    nxt = []
                    for i in range(0, len(part_maxes), 2):
                        cm = stats.tile([P, 1], F32, tag=f"cmb{len(part_maxes)}_{i}",
                                        bufs=2)
                        nc.vector.tensor_max(cm[:], part_maxes[i][:],
                                             part_maxes[i + 1][:])
                        nxt.append(cm)
                    part_maxes = nxt
                amax = part_maxes[0]
                # xs = max(amax, eps) * (1/224); quant scale is exactly 1/xs
                xs = xspool.tile([P, 1], F32, tag="xs")
                nc.vector.tensor_scalar(
                    out=xs[:], in0=amax[:],
                    scalar1=1e-10, scalar2=1.0 / 224.0,
                    op0=mybir.AluOpType.max, op1=mybir.AluOpType.mult,
                )
                xs_tiles.append(xs)
                inv = stats.tile([P, 1], F32, tag="inv")
                prev_inv_inst = nc.vector.reciprocal(out=inv[:], in_=xs[:])

            # quantize in halves: the transposes of half A start while
            # half B is still quantizing on ACT
            nq = 2
            KQP = K // nq
            ks_per = KQP // P
            xq_parts = []
            for h in range(nq):
                xq_h = xqpool.tile([P, KQP], FP8, tag=f"xq{nq}_{h}", bufs=2)
                nc.scalar.activation(
                    out=xq_h[:], in_=xt[:, h * KQP:(h + 1) * KQP],
                    func=mybir.ActivationFunctionType.Copy, scale=inv[:],
                )
                xq_parts.append(xq_h)

            def xq_chunk(ks):
                return xq_parts[ks // ks_per][:, (ks % ks_per) * P:(ks % ks_per + 1) * P]

            # transpose x_q into [K, M] layout via PE transpose matmuls;
            # evict each 8-ksub group right after its matmuls so the GEMM's
            # j=0..3 can start as soon as the first group lands in SBUF
            xqt_groups = []
            for half in range(2):
                if USE_IS_TRANSPOSE:
                    # fp8 transpose mode writes elements on a 2-byte step
                    ptr = psum_tr.tile([P, 16, 2 * P], FP8, tag="ptr")
                    ptr_view = ptr[:, :, 0:2 * P:2]
                else:
                    ptr = psum_tr.tile([P, 16, P], F32, tag="ptr")
                    ptr_view = ptr[:]
                for g in range(2):
                    for i in range(8):
                        nc.tensor.matmul(
                            out=ptr_view[:, 8 * g + i, :],
                            lhsT=xq_chunk(half * 16 + 8 * g + i),
                            rhs=ident[:],
                            start=True, stop=True,
                            is_transpose=USE_IS_TRANSPOSE,
                        )
                    xqt_g = xqtpool.tile([P, 8, P], FP8, tag=f"xqt{2 * half + g}")
                    xqt_groups.append(xqt_g)
                    nc.scalar.copy(out=xqt_g[:], in_=ptr_view[:, 8 * g:8 * g + 8, :])
            xqt_tiles.append(xqt_groups)

            if mt >= 1:
                gemm_block(mt - 1, 1, phase1=True)
            gemm_block(mt, 0, phase1=True)

        gemm_block(M_TILES - 1, 1, phase1=True)

        # ---- phase 2: pure fp8 DoubleRow GEMM over the remaining N-blocks
        for nb in range(NB_PHASE1, N_BLKS):
            if nb + 1 < N_BLKS:
                issue_wslab(nb + 1, nc.sync)
            for mt in range(M_TILES):
                gemm_block(mt, nb)

    nc.compile()
    return nc


def _get_program():
    if "nc" not in _PROGRAM_CACHE:
        _PROGRAM_CACHE["nc"] = _build_program()
    return _PROGRAM_CACHE["nc"]


def _run_sharded(x, weight, weight_scales, bias, trace=False):
    x = np.asarray(x).astype(ml_dtypes.bfloat16, copy=False)
    weight = np.asarray(weight, dtype=np.float32)
    weight_scales = np.asarray(weight_scales, dtype=np.float32)
    bias = np.asarray(bias, dtype=np.float32)

    # host-side sharding / layout only:
    # wt[nb, p, ksub, n] = weight[nb*512 + n, ksub*128 + p], re-encoded to
    # fp8 e4m3 (lossless: the reference weights are fp8-round-tripped values)
    wt = np.ascontiguousarray(
        weight.T.reshape(K_SUBS, P, N_BLKS, N_BLK).transpose(2, 1, 0, 3)
    ).astype(ml_dtypes.float8_e4m3)
    in_maps = []
    for c in range(NCORES):
        in_maps.append({
            "x": np.ascontiguousarray(x[c * M_SHARD:(c + 1) * M_SHARD]),
            "wt": wt,
            "ws": weight_scales,
            "bias": bias,
        })

    nc = _get_program()
    res = run_bass_kernel_spmd(nc, in_maps, core_ids=list(range(NCORES)), trace=trace)
    out = np.concatenate([res.results[c]["out"] for c in range(NCORES)], axis=0)
    return out, res.exec_time_ns


def kernel(x, weight, weight_scales, bias):
    out, _ = _run_sharded(x, weight, weight_scales, bias,
                          trace=bool(os.environ.get("KERNEL_TRACE")))
    return out
